# revision 38
# baseline (speedup 1.0000x reference)
"""CostGlobalEncoder TRN2 kernel: conv3x3(324->128) + global HW x HW attention
+ proj + FFN, data-parallel over batch N=8 across 8 NeuronCores.

Self-contained: hardcodes shapes N=8, D=128, H=48, W=64 (HW=3072).

Structure (per core, one batch sample):
  - conv feeds q; S = k^T q per 128-key j-tile; exp on ScalarE writes fp8
    e-tiles; the proj weight half Wp0 is folded into V on the host
    (Wp0 @ (V E) == (Wp0 V) @ E), so the fp8 DoubleRow AV accumulation
    directly produces the projected attention term and the per-pair
    boundary needs no proj matmul at all.
  - softmax denominators via fp8 ones-matmuls col-packed 4-wide into one
    PSUM bank; denominator merge+broadcast via one mask matmul; the
    reciprocal uses the fast custom-DVE approximation (~5x faster than
    InstReciprocal), keeping the boundary chain short.
  - Pair boundaries are software-pipelined: the merge/recip/normalize
    chain of pair p-1 is injected into j=0..3 of pair p's j-loop with the
    next conv's matmuls emitted first each j, so the PE never idles long
    enough for the HAM clock gate to drop back to 1.2 GHz.
  - All FFN gelus run at the tail (one Act table switch), pipelined
    wf1 -> gelu -> wf2 -> out with the PSUM->SBUF out-copies alternating
    between ScalarE (identity-matmul residual) and VectorE (tensor_add
    residual), and output DMA alternating between the two HWDGE rings.
"""
import sys
sys.path.insert(0, '/opt/trn_rl_repo')

import numpy as np
import ml_dtypes

import concourse.bass as bass
import concourse.tile as tile
from concourse import mybir
from concourse.bass_utils import run_bass_kernel_spmd

N, D, H, W = 8, 128, 48, 64
HW = H * W                    # 3072
CIN = 324                     # corr channels
KC = 108                      # conv contraction chunk (324 = 3*108)
NT = 6                        # i-tiles of 512 positions
NP = NT // 2                  # i-tile pairs
TI = 512                      # positions per i-tile
RT = TI // W                  # 8 rows per i-tile
NJ = HW // 128                # 24 j-tiles
NJP = NJ // 2                 # 12 j-tile pairs (fp8 DoubleRow)
SCALE = float(D) ** -0.5
EBIAS = -3.0                  # exp bias keeps fp8 e-values < 240 (TRN e4m3 inf)

F32 = mybir.dt.float32
BF16 = mybir.dt.bfloat16
F8 = mybir.dt.float8e4
AF = mybir.ActivationFunctionType
DR = mybir.MatmulPerfMode.DoubleRow


def _split_multi_waits(nc, max_waits=1):
    """walrus setupSyncWait rejects instructions with several sem-waits;
    hoist extras onto preceding same-engine NOPs (engines run in order)."""
    for fn in nc.m.functions:
        for blk in fn.blocks:
            insts = blk.instructions
            i = 0
            while i < len(insts):
                inst = insts[i]
                si = inst.sync_info
                if si is not None and si.on_wait and len(si.on_wait) > max_waits:
                    waits = list(si.on_wait)
                    extra, keep = waits[:-max_waits], waits[-max_waits:]
                    nops = []
                    while extra:
                        chunk, extra = extra[:max_waits], extra[max_waits:]
                        nop = mybir.InstNoOp(
                            name=f"waitsplit-{nc.next_id()}", ins=[], outs=[])
                        nop.engine = inst.engine
                        nop.sync_info = mybir.SyncInfo(on_wait=chunk, on_update=[])
                        nops.append(nop)
                    inst.sync_info = mybir.SyncInfo(
                        on_wait=keep, on_update=list(si.on_update))
                    blk.instructions = insts = insts[:i] + nops + insts[i:]
                    i += len(nops)
                i += 1


def build_nc(with_bias=True):
    nc = bass.Bass()
    corr = nc.declare_dram_parameter("corr", [CIN, HW], BF16, isOutput=False)
    k_in = nc.declare_dram_parameter("k", [D, HW], BF16, isOutput=False)
    wvT2 = nc.declare_dram_parameter("wvT2", [128, NJP, 2, D], F8, isOutput=False)
    wskT = nc.declare_dram_parameter("wskT", [KC, 27, D], BF16, isOutput=False)
    b_sk = nc.declare_dram_parameter("b_sk", [1, D], BF16, isOutput=False)
    wp1T = nc.declare_dram_parameter("wp1T", [D, D], BF16, isOutput=False)
    b_proj = nc.declare_dram_parameter("b_proj", [1, D], BF16, isOutput=False)
    wf1T = nc.declare_dram_parameter("wf1T", [D, D], BF16, isOutput=False)
    b_f1 = nc.declare_dram_parameter("b_f1", [D, 1], F32, isOutput=False)
    wf2T = nc.declare_dram_parameter("wf2T", [D, D], BF16, isOutput=False)
    b_f2 = nc.declare_dram_parameter("b_f2", [1, D], BF16, isOutput=False)
    ident = nc.declare_dram_parameter("ident", [D, D], BF16, isOutput=False)
    out = nc.declare_dram_parameter("out", [D, HW], F32, isOutput=True)

    HH = H // 2  # corr DMA half-chunk rows

    with tile.TileContext(nc) as tc:
        with (
            tc.tile_pool(name="const", bufs=1) as cpool,
            tc.tile_pool(name="stage", bufs=2) as spool,
            tc.tile_pool(name="work", bufs=3) as wpool,
            tc.tile_pool(name="qpool", bufs=4) as qpool,
            tc.tile_pool(name="xpool", bufs=7) as xpool,
            tc.tile_pool(name="epool", bufs=9) as epool,
            tc.tile_pool(name="ps_s", bufs=2, space="PSUM") as ps_s,
            tc.tile_pool(name="ps_av", bufs=2, space="PSUM") as ps_av,
            tc.tile_pool(name="ps_c", bufs=1, space="PSUM") as ps_cp,
            tc.tile_pool(name="ps_m", bufs=1, space="PSUM") as ps_mp,
        ):
            # ---- memsets first so nothing queues behind DMA descriptors.
            # warm goes on GpSimd (its init finishes first and its queue is
            # empty) so the PE warm-up can start ~2us earlier. ----
            warm = cpool.tile([128, 128], BF16)
            nc.gpsimd.memset(warm[:], 0.0)
            ones_f8 = cpool.tile([128, 1], F8)
            nc.gpsimd.memset(ones_f8[:], 1.0)
            ones_row = cpool.tile([1, TI], BF16)
            nc.gpsimd.memset(ones_row[:], 1.0)
            ebias_sb = cpool.tile([128, 1], F32)
            nc.gpsimd.memset(ebias_sb[:], EBIAS)
            corr_pad = []
            for c in range(3):
                cp = cpool.tile([KC, H + 2, W + 2], BF16, name=f"corr_pad{c}")
                nc.vector.memset(cp[:, 0, :], 0.0)
                nc.vector.memset(cp[:, H + 1, :], 0.0)
                nc.vector.memset(cp[:, 1:H + 1, 0:1], 0.0)
                nc.vector.memset(cp[:, 1:H + 1, W + 1:W + 2], 0.0)
                corr_pad.append(cp)
            # mask[r, ii, :] = 1 iff row r holds an ii-denominator partial
            # (ii0 partials live in rows {0,32}, ii1 in rows {64,96})
            mask = cpool.tile([128, 2, 128], BF16)
            nc.gpsimd.memset(mask[:], 0.0)
            for t in range(2):
                for ii in range(2):
                    r = 64 * ii + 32 * t
                    nc.vector.memset(mask[r:r + 1, ii, :], 1.0)

            # ---- input DMAs split across the two HWDGE rings: corr halves
            # on the Sync ring, weights/k/v on the Scalar ring, so the conv
            # critical path and the attention operands load in parallel ----
            wskT_sb = cpool.tile([KC, 27, D], BF16)
            k_sb = cpool.tile([D, HW], BF16)
            wvT2_sb = cpool.tile([128, NJP, 2, D], F8)
            def stg_copy(c, half):
                # second-half staging copies run on GpSimd: their DMAs
                # land mid-loop and would head-of-line block the DVE in
                # front of the conv q/resid copies
                stg = spool.tile([KC, HH * W], BF16, name="stg", bufs=3)
                nc.sync.dma_start(stg[:], corr[c * KC:(c + 1) * KC,
                                               half * HH * W:(half + 1) * HH * W])
                r0 = 1 + half * HH
                eng = nc.vector if half == 0 else nc.gpsimd
                eng.tensor_copy(
                    corr_pad[c][:, r0:r0 + HH, 1:W + 1],
                    stg.rearrange("p (h w) -> p h w", h=HH))

            # the conv's two input streams (weights + first corr halves)
            # interleave on the Sync ring in consumption order — the two
            # HWDGE rings share HBM bandwidth, so priority must be global,
            # not per-ring. k/vT2 ride the Scalar ring: S(0)/S(1) only
            # need the first k column-chunk, and the AV accumulation
            # starts at loop j=1, so vT2 beats k1/k2.
            nc.sync.dma_start(wskT_sb[:, 0:9, :], wskT[:, 0:9, :])
            stg_copy(0, 0)
            nc.sync.dma_start(wskT_sb[:, 9:18, :], wskT[:, 9:18, :])
            stg_copy(1, 0)
            nc.sync.dma_start(wskT_sb[:, 18:27, :], wskT[:, 18:27, :])
            stg_copy(2, 0)
            nc.scalar.dma_start(k_sb[:, 0:1024], k_in[:, 0:1024])
            nc.scalar.dma_start(wvT2_sb[:], wvT2[:])
            nc.scalar.dma_start(k_sb[:, 1024:2048], k_in[:, 1024:2048])
            nc.scalar.dma_start(k_sb[:, 2048:HW], k_in[:, 2048:HW])
            # preload the exp table set during the DMA wait (after the
            # Scalar-ring triggers so the table DMA doesn't delay them)
            tdummy = cpool.tile([1, 16], BF16)
            nc.scalar.activation(tdummy[:], warm[0:1, 0:16], AF.Exp)
            for c in range(3):
                stg_copy(c, 1)
            wp1T_sb = cpool.tile([D, D], BF16)
            nc.gpsimd.dma_start(wp1T_sb[:], wp1T[:])
            wf1T_sb = cpool.tile([D, D], BF16)
            nc.gpsimd.dma_start(wf1T_sb[:], wf1T[:])
            wf2T_sb = cpool.tile([D, D], BF16)
            nc.gpsimd.dma_start(wf2T_sb[:], wf2T[:])
            ident_sb = cpool.tile([D, D], BF16)
            nc.gpsimd.dma_start(ident_sb[:], ident[:])
            b_sk_sb = cpool.tile([1, D], BF16)
            nc.gpsimd.dma_start(b_sk_sb[:], b_sk[:])
            b_proj_sb = cpool.tile([1, D], BF16)
            nc.gpsimd.dma_start(b_proj_sb[:], b_proj[:])
            b_f1_sb = cpool.tile([D, 1], F32)
            nc.gpsimd.dma_start(b_f1_sb[:], b_f1[:])
            b_f2_sb = cpool.tile([1, D], BF16)
            nc.gpsimd.dma_start(b_f2_sb[:], b_f2[:])

            # HAM warm-up: keep PE busy while the first corr half lands
            ps_w = ps_cp.tile([128, 128], F32, name="ps_w", tag="c")
            for _ in range(54):
                nc.tensor.matmul(ps_w[:], warm[:], warm[:],
                                 start=True, stop=True)
            # dependency-free filler target for HAM-hold matmuls inside
            # the DMA-paced prologue conv ("m" slot is idle until j=12)
            ps_fill = ps_mp.tile([128, 128], F32, name="ps_fill", tag="m")

            def fill_pe(n):
                for _ in range(n):
                    nc.tensor.matmul(ps_fill[:], warm[:], warm[:],
                                     start=True, stop=True)

            def conv_gen(p, filler=False):
                """3x3 conv for i-tiles (2p, 2p+1), one [D,TI] psum bank,
                ii-serial. Also computes resid2 = conv + b_sk + b_proj
                + Wp1 @ q (the proj residual half, so the boundary only
                adds the normalized attention term). Yields None after
                each PE matmul, then yields [(resid2, q), ...] forever.
                filler=True (prologue pair only) emits dependency-free
                matmuls between weight chunks so the HAM clock gate never
                sees an idle window while the wskT DMA streams in."""
                outs = []
                for ii in range(2):
                    ps_c = ps_cp.tile([D, TI], F32, name="ps_c", tag="c")
                    y0 = (2 * p + ii) * RT
                    for c in range(3):
                        for t in range(9):
                            dy, dx = t // 3, t % 3
                            nc.tensor.matmul(
                                ps_c[:],
                                wskT_sb[:, c * 9 + t, :],
                                corr_pad[c][:, y0 + dy:y0 + dy + RT,
                                            dx:dx + W],
                                start=(c == 0 and t == 0),
                                stop=(c == 2 and t == 8 and not with_bias))
                            yield None
                        if filler and c < 2:
                            fill_pe(8)
                    if with_bias:
                        nc.tensor.matmul(ps_c[:], b_sk_sb[:], ones_row[:],
                                         start=False, stop=True)
                        yield None
                    q = qpool.tile([D, TI], BF16, name="q", tag="q")
                    nc.vector.tensor_copy(q[:], ps_c[:])
                    resid = qpool.tile([D, TI], F32, name="resid", tag="r")
                    nc.vector.tensor_copy(resid[:], ps_c[:])
                    yield None
                    yield None   # extra slack so the PE never waits on the
                    yield None   # DVE q-copy before the ps_r matmul
                    ps_r = ps_cp.tile([D, TI], F32, name="ps_r", tag="c")
                    nc.tensor.matmul(ps_r[:], wp1T_sb[:], q[:],
                                     start=True, stop=not with_bias)
                    if with_bias:
                        nc.tensor.matmul(ps_r[:], b_proj_sb[:], ones_row[:],
                                         start=False, stop=True)
                    yield None
                    resid2 = qpool.tile([D, TI], F32, name="resid2", tag="r2")
                    nc.vector.tensor_add(resid2[:], ps_r[:], resid[:])
                    outs.append((resid2, q))
                    yield None
                while True:
                    yield outs

            def run_conv(gen):
                while True:
                    r = next(gen)
                    if r is not None:
                        return r

            def s_pair(j, qs):
                t = ps_s.tile([128, 2, TI], F32, name="ps_sj", tag="s")
                for ii in range(2):
                    nc.tensor.matmul(t[:, ii, :],
                                     k_sb[:, j * 128:(j + 1) * 128],
                                     qs[ii][:], start=True, stop=True)
                return t

            def ones_half(ps_m, e2t, stop):
                """denominator partial sums for one e-tile (2 j-tiles):
                fp8 ones-matmuls col-packed 4-wide; ii0 -> rows {0,32},
                ii1 -> {64,96}. ps_m was zero-initialized by a start=True
                matmul, so always accumulate. The caller sets stop on the
                last e-tile so every row's accumulation chain closes."""
                for ii in range(2):
                    for jj in range(2):
                        r = 64 * ii + 32 * jj
                        nc.tensor.matmul(
                            ps_m[r:r + 1, :], ones_f8[:, 0:1],
                            e2t[:, jj, ii, :],
                            start=False, stop=stop,
                            skip_group_check=True,
                            tile_position=(0, r))

            def new_ps_m():
                """fresh denominator bank, zeroed by a 0-weights matmul so
                the mask-merge never touches stale PSUM garbage."""
                ps_m = ps_mp.tile([128, TI], F32, name="ps_m", tag="m")
                nc.tensor.matmul(ps_m[:], warm[:], k_sb[:, 0:TI],
                                 start=True, stop=True)
                return ps_m

            def merge_den(m4, ii):
                """broadcast mask-merged denominator into a psum bank."""
                ps_b = ps_mp.tile([128, TI], F32, name="ps_b", tag="m")
                nc.tensor.matmul(ps_b[:], mask[:, ii, :], m4[:],
                                 start=True, stop=True)
                return ps_b

            def recip_den(ps_b):
                rb = wpool.tile([128, TI], F32, name="rb", tag="rb")
                nc.vector.reciprocal_approx_fast(rb[:], ps_b[:])
                return rb

            def finish_x(av_sb, rb, resid2):
                t = wpool.tile([D, TI], F32, name="t", tag="t")
                nc.vector.tensor_mul(t[:], av_sb[:], rb[:])
                x_bf = xpool.tile([D, TI], BF16, name="x_bf", tag="xb")
                nc.vector.tensor_add(x_bf[:], t[:], resid2[:])
                return x_bf

            def ffn1(x_bf):
                ps_f1 = ps_av.tile([D, TI], F32, name="ps_f1", tag="av")
                nc.tensor.matmul(ps_f1[:], wf1T_sb[:], x_bf[:],
                                 start=True, stop=True)
                return ps_f1

            def gelu_of(ps_f1):
                h1 = wpool.tile([D, TI], BF16, name="h1", tag="h1")
                nc.scalar.activation(h1[:], ps_f1[:], AF.Gelu, bias=b_f1_sb[:])
                return h1

            def ffn2(x_bf, h1, use_ident):
                ps_f2 = ps_s.tile([D, TI], F32, name="ps_f2", tag="s")
                if use_ident:
                    nc.tensor.matmul(ps_f2[:], ident_sb[:], x_bf[:],
                                     start=True, stop=False)
                    nc.tensor.matmul(ps_f2[:], wf2T_sb[:], h1[:],
                                     start=False, stop=not with_bias)
                else:
                    nc.tensor.matmul(ps_f2[:], wf2T_sb[:], h1[:],
                                     start=True, stop=not with_bias)
                if with_bias:
                    nc.tensor.matmul(ps_f2[:], b_f2_sb[:], ones_row[:],
                                     start=False, stop=True)
                return ps_f2

            def out_tile(ps_f2, x_bf, i, use_ident):
                """PSUM->SBUF out copy + residual, then DMA. Even tiles got
                the residual via the identity matmul (ScalarE plain copy);
                odd tiles add it here on VectorE. DMA alternates rings."""
                o = wpool.tile([D, TI], F32, name="o", tag="o")
                if use_ident:
                    nc.scalar.copy(o[:], ps_f2[:])
                else:
                    nc.vector.tensor_add(o[:], ps_f2[:], x_bf[:])
                # even tiles exit via the Scalar ring right after their Act
                # copy; odd tiles via the Sync ring (idle at the tail)
                eng = nc.scalar if use_ident else nc.sync
                eng.dma_start(out[:, i * TI:(i + 1) * TI], o[:])

            # ---- prologue: conv pair 0 + S(0), S(1) ----
            rq = run_conv(conv_gen(0, filler=True))
            fill_pe(4)
            qpair = [rq[0][1], rq[1][1]]
            ps_s0 = s_pair(0, qpair)
            ps_s1 = s_pair(1, qpair)

            xs = [None] * NT
            prev = None              # (ps_m, ps_a, rq) of pair p-1
            for p in range(NP):
                last = p == NP - 1
                cgen = conv_gen(p + 1) if not last else None
                cdone = None
                qnext = None
                # boundary leftovers from pair p-1, injected into j=0..3 of
                # this pair's loop; the chain never blocks the PE because
                # conv matmuls are emitted first within each j.
                inject = []
                if prev is not None:
                    pm_ps, pav_ps, prq = prev
                    st = {}

                    def step0(pm_ps=pm_ps, pav_ps=pav_ps, st=st):
                        m4 = wpool.tile([128, TI], BF16, name="m4", tag="m4")
                        nc.vector.tensor_copy(m4[:], pm_ps[:])
                        avs = []
                        for ii in range(2):
                            a = qpool.tile([D, TI], F32, name="av_sb",
                                           tag="avs")
                            nc.vector.tensor_copy(a[:], pav_ps[ii][:])
                            avs.append(a)
                        st['m4'] = m4
                        st['av'] = avs
                        st['b0'] = merge_den(m4, 0)

                    def step1(st=st):
                        st['rb0'] = recip_den(st['b0'])
                        st['b1'] = merge_den(st['m4'], 1)

                    def step2(st=st, prq=prq, pp=p):
                        st['rb1'] = recip_den(st['b1'])
                        xs[2 * (pp - 1)] = finish_x(
                            st['av'][0], st['rb0'], prq[0][0])

                    def step3(st=st, prq=prq, pp=p):
                        xs[2 * (pp - 1) + 1] = finish_x(
                            st['av'][1], st['rb1'], prq[1][0])

                    inject = [step0, step1, step2, step3]

                ps_m = None
                ps_sj, ps_snx = ps_s0, ps_s1
                ps_a = None
                e2 = None
                e2_hist = []
                for j in range(NJ):
                    jp, jj = j // 2, j % 2
                    if jj == 0:
                        e2 = epool.tile([128, 2, 2, TI], F8, name="e2",
                                        tag="e")
                        e2_hist.append(e2)
                    # Act: the pacing instruction
                    nc.scalar.activation(e2[:, jj, :, :], ps_sj[:], AF.Exp,
                                         scale=SCALE, bias=ebias_sb[:])
                    # conv for the next pair: emitted first so the PE has
                    # dependency-free work while boundary copies settle
                    if cgen is not None:
                        for _ in range(3):
                            next(cgen)
                    # S prefetch / next-pair S tiles
                    if j < NJ - 2:
                        ps_sj, ps_snx = ps_snx, s_pair(j + 2, qpair)
                    elif j == NJ - 2:
                        ps_sj = ps_snx
                        if not last:
                            cdone = run_conv(cgen)
                            cgen = None
                            qnext = [cdone[0][1], cdone[1][1]]
                            ps_s0 = s_pair(0, qnext)
                    else:
                        if not last:
                            ps_s1 = s_pair(1, qnext)
                    # denominator groups: deferred so the 'm' bank is free
                    # of the previous boundary's merge chain; the last
                    # e-tile's half runs at the boundary (only 4 matmuls
                    # gated on the final exp)
                    if j in (12, 14, 16, 18, 20):
                        g4 = (j - 12) // 2
                        if ps_m is None:
                            ps_m = new_ps_m()
                        ones_half(ps_m, e2_hist[2 * g4], False)
                        ones_half(ps_m, e2_hist[2 * g4 + 1], False)
                    elif j == NJ - 2:
                        ones_half(ps_m, e2_hist[10], False)
                    # boundary injects (before AV so the previous pair's
                    # psum readers are emitted before this pair reuses the
                    # ps_av slots)
                    if inject and j < 4:
                        inject.pop(0)()
                    if jj == 1:
                        if jp == 0:
                            ps_a = [ps_av.tile([D, TI], F32,
                                               name=f"ps_a{ii}", tag="av")
                                    for ii in range(2)]
                        for ii in range(2):
                            nc.tensor.matmul(
                                ps_a[ii][:], wvT2_sb[:, jp, :, :],
                                e2[:, :, ii, :],
                                start=(jp == 0), stop=(jp == NJP - 1),
                                perf_mode=DR)
                # ---- boundary: finish denominators; evacuation and
                # normalize happen inside the next pair's loop (or tail) ----
                ones_half(ps_m, e2_hist[11], True)
                prev = (ps_m, ps_a, rq)
                if not last:
                    rq = cdone
                    qpair = qnext

            # ---- tail: pair-2 normalize + all 6 FFNs (gelus after all
            # exps: one table switch, issued first on Act so the table DMA
            # overlaps the merge/recip/normalize chain) ----
            pm_ps, pav_ps, rq2 = prev
            m4 = wpool.tile([128, TI], BF16, name="m4", tag="m4")
            nc.vector.tensor_copy(m4[:], pm_ps[:])
            av_t = []
            for ii in range(2):
                a = qpool.tile([D, TI], F32, name="av_sb", tag="avs")
                nc.vector.tensor_copy(a[:], pav_ps[ii][:])
                av_t.append(a)
            f1_0 = ffn1(xs[0])
            h0 = gelu_of(f1_0)
            f1_1 = ffn1(xs[1])
            b0 = merge_den(m4, 0)
            rb0 = recip_den(b0)
            h1 = gelu_of(f1_1)
            b1 = merge_den(m4, 1)
            rb1 = recip_den(b1)
            xs[4] = finish_x(av_t[0], rb0, rq2[0][0])
            xs[5] = finish_x(av_t[1], rb1, rq2[1][0])
            f2_0 = ffn2(xs[0], h0, True)
            out_tile(f2_0, xs[0], 0, True)
            f1_2 = ffn1(xs[2])
            h2 = gelu_of(f1_2)
            f2_1 = ffn2(xs[1], h1, False)
            out_tile(f2_1, xs[1], 1, False)
            f1_3 = ffn1(xs[3])
            h3 = gelu_of(f1_3)
            f2_2 = ffn2(xs[2], h2, True)
            out_tile(f2_2, xs[2], 2, True)
            f1_4 = ffn1(xs[4])
            h4 = gelu_of(f1_4)
            f2_3 = ffn2(xs[3], h3, False)
            out_tile(f2_3, xs[3], 3, False)
            f1_5 = ffn1(xs[5])
            h5 = gelu_of(f1_5)
            f2_4 = ffn2(xs[4], h4, True)
            out_tile(f2_4, xs[4], 4, True)
            f2_5 = ffn2(xs[5], h5, False)
            out_tile(f2_5, xs[5], 5, False)

    # populate .instr bytes for extended-inst ISA subclasses (the custom
    # DVE reciprocal) — raw Bass skips this Bacc.compile() pass and the
    # NEFF compiler fails with "ISA wrong length" without it
    mybir.codegen_inst_isa_subclasses(nc)
    _split_multi_waits(nc)
    return nc


_NC = {}


def _get_nc(with_bias=True):
    if with_bias not in _NC:
        _NC[with_bias] = build_nc(with_bias)
    return _NC[with_bias]


def _prep_core(corr, k, v, w_sk, b_sk, w_proj, b_proj, w_ffn1, b_ffn1,
               w_ffn2, b_ffn2):
    bf = ml_dtypes.bfloat16
    f8 = ml_dtypes.float8_e4m3
    wskT = np.empty((KC, 27, D), dtype=bf)
    for c in range(3):
        for t in range(9):
            dy, dx = t // 3, t % 3
            wskT[:, c * 9 + t, :] = \
                w_sk[:, c * KC:(c + 1) * KC, dy, dx].T.astype(bf)
    # fold the attention half of the proj into v:
    # w_proj @ concat(av, resid) = Wp0 @ av + Wp1 @ resid, and
    # Wp0 @ (V E) == (Wp0 V) @ E, so quantize Wp0 V to fp8 instead of V.
    wp = w_proj.reshape(D, 2 * D)
    wv = wp[:, :D] @ v.reshape(D, HW)                     # [D, HW] f32
    wvT = wv.T.reshape(NJ, 128, D).transpose(1, 0, 2)
    wvT2 = np.ascontiguousarray(wvT).astype(f8).reshape(128, NJP, 2, D)
    return {
        "corr": corr.reshape(CIN, HW).astype(bf),
        "k": k.reshape(D, HW).astype(bf),
        "wvT2": wvT2,
        "wskT": wskT,
        "b_sk": b_sk.reshape(1, D).astype(bf),
        "wp1T": np.ascontiguousarray(wp[:, D:].T).astype(bf),
        "b_proj": b_proj.reshape(1, D).astype(bf),
        "wf1T": np.ascontiguousarray(w_ffn1.reshape(D, D).T).astype(bf),
        "b_f1": b_ffn1.reshape(D, 1).astype(np.float32),
        "wf2T": np.ascontiguousarray(w_ffn2.reshape(D, D).T).astype(bf),
        "b_f2": b_ffn2.reshape(1, D).astype(bf),
        "ident": np.eye(D, dtype=bf),
    }


def make_in_maps(corr, k, v, w_sk, b_sk, w_proj, b_proj, w_ffn1, b_ffn1,
                 w_ffn2, b_ffn2):
    corr = np.asarray(corr, dtype=np.float32)
    k = np.asarray(k, dtype=np.float32)
    v = np.asarray(v, dtype=np.float32)
    return [
        _prep_core(corr[i], k[i], v[i], np.asarray(w_sk, np.float32),
                   np.asarray(b_sk, np.float32),
                   np.asarray(w_proj, np.float32),
                   np.asarray(b_proj, np.float32),
                   np.asarray(w_ffn1, np.float32),
                   np.asarray(b_ffn1, np.float32),
                   np.asarray(w_ffn2, np.float32),
                   np.asarray(b_ffn2, np.float32))
        for i in range(N)
    ]


def kernel(corr, k, v, w_sk, b_sk, w_proj, b_proj, w_ffn1, b_ffn1,
           w_ffn2, b_ffn2):
    with_bias = bool(np.any(np.asarray(b_proj)) or np.any(np.asarray(b_ffn2))
                     or np.any(np.asarray(b_sk)))
    nc = _get_nc(with_bias)
    in_maps = make_in_maps(corr, k, v, w_sk, b_sk, w_proj, b_proj,
                           w_ffn1, b_ffn1, w_ffn2, b_ffn2)
    res = run_bass_kernel_spmd(nc, in_maps, list(range(N)))
    out = np.stack([res.results[i]["out"].reshape(D, H, W) for i in range(N)])
    return out.astype(np.float32)


# revision 40
# speedup vs baseline: 1.0075x; 1.0075x over previous
"""CostGlobalEncoder TRN2 kernel: conv3x3(324->128) + global HW x HW attention
+ proj + FFN, data-parallel over batch N=8 across 8 NeuronCores.

Self-contained: hardcodes shapes N=8, D=128, H=48, W=64 (HW=3072).

Structure (per core, one batch sample):
  - conv feeds q; S = k^T q per 128-key j-tile; exp on ScalarE writes fp8
    e-tiles; the proj weight half Wp0 is folded into V on the host
    (Wp0 @ (V E) == (Wp0 V) @ E), so the fp8 DoubleRow AV accumulation
    directly produces the projected attention term and the per-pair
    boundary needs no proj matmul at all.
  - softmax denominators via fp8 ones-matmuls col-packed 4-wide into one
    PSUM bank; denominator merge+broadcast via one mask matmul; the
    reciprocal uses the fast custom-DVE approximation (~5x faster than
    InstReciprocal), keeping the boundary chain short.
  - Pair boundaries are software-pipelined: the merge/recip/normalize
    chain of pair p-1 is injected into j=0..3 of pair p's j-loop with the
    next conv's matmuls emitted first each j, so the PE never idles long
    enough for the HAM clock gate to drop back to 1.2 GHz.
  - All FFN gelus run at the tail (one Act table switch), pipelined
    wf1 -> gelu -> wf2 -> out with the PSUM->SBUF out-copies alternating
    between ScalarE (identity-matmul residual) and VectorE (tensor_add
    residual), and output DMA alternating between the two HWDGE rings.
"""
import sys
sys.path.insert(0, '/opt/trn_rl_repo')

import numpy as np
import ml_dtypes

import concourse.bass as bass
import concourse.tile as tile
from concourse import mybir
from concourse.bass_utils import run_bass_kernel_spmd

N, D, H, W = 8, 128, 48, 64
HW = H * W                    # 3072
CIN = 324                     # corr channels
KC = 108                      # conv contraction chunk (324 = 3*108)
NT = 6                        # i-tiles of 512 positions
NP = NT // 2                  # i-tile pairs
TI = 512                      # positions per i-tile
RT = TI // W                  # 8 rows per i-tile
NJ = HW // 128                # 24 j-tiles
NJP = NJ // 2                 # 12 j-tile pairs (fp8 DoubleRow)
SCALE = float(D) ** -0.5
EBIAS = -3.0                  # exp bias keeps fp8 e-values < 240 (TRN e4m3 inf)

F32 = mybir.dt.float32
BF16 = mybir.dt.bfloat16
F8 = mybir.dt.float8e4
AF = mybir.ActivationFunctionType
DR = mybir.MatmulPerfMode.DoubleRow


def _split_multi_waits(nc, max_waits=1):
    """walrus setupSyncWait rejects instructions with several sem-waits;
    hoist extras onto preceding same-engine NOPs (engines run in order)."""
    for fn in nc.m.functions:
        for blk in fn.blocks:
            insts = blk.instructions
            i = 0
            while i < len(insts):
                inst = insts[i]
                si = inst.sync_info
                if si is not None and si.on_wait and len(si.on_wait) > max_waits:
                    waits = list(si.on_wait)
                    extra, keep = waits[:-max_waits], waits[-max_waits:]
                    nops = []
                    while extra:
                        chunk, extra = extra[:max_waits], extra[max_waits:]
                        nop = mybir.InstNoOp(
                            name=f"waitsplit-{nc.next_id()}", ins=[], outs=[])
                        nop.engine = inst.engine
                        nop.sync_info = mybir.SyncInfo(on_wait=chunk, on_update=[])
                        nops.append(nop)
                    inst.sync_info = mybir.SyncInfo(
                        on_wait=keep, on_update=list(si.on_update))
                    blk.instructions = insts = insts[:i] + nops + insts[i:]
                    i += len(nops)
                i += 1


def build_nc(with_bias=True):
    nc = bass.Bass()
    corr = nc.declare_dram_parameter("corr", [CIN, HW], BF16, isOutput=False)
    k_in = nc.declare_dram_parameter("k", [D, HW], BF16, isOutput=False)
    wvT2 = nc.declare_dram_parameter("wvT2", [128, NJP, 2, D], F8, isOutput=False)
    wskT = nc.declare_dram_parameter("wskT", [KC, 27, D], BF16, isOutput=False)
    b_sk = nc.declare_dram_parameter("b_sk", [1, D], BF16, isOutput=False)
    wp1T = nc.declare_dram_parameter("wp1T", [D, D], BF16, isOutput=False)
    b_proj = nc.declare_dram_parameter("b_proj", [1, D], BF16, isOutput=False)
    wf1T = nc.declare_dram_parameter("wf1T", [D, D], BF16, isOutput=False)
    b_f1 = nc.declare_dram_parameter("b_f1", [D, 1], F32, isOutput=False)
    wf2T = nc.declare_dram_parameter("wf2T", [D, D], BF16, isOutput=False)
    b_f2 = nc.declare_dram_parameter("b_f2", [1, D], BF16, isOutput=False)
    ident = nc.declare_dram_parameter("ident", [D, D], BF16, isOutput=False)
    out = nc.declare_dram_parameter("out", [D, HW], F32, isOutput=True)

    HH = H // 2  # corr DMA half-chunk rows

    with tile.TileContext(nc) as tc:
        # two pools only (slots are keyed per tag/name with explicit bufs,
        # so semantics match the old 10-pool layout) — every pool exit
        # costs an all-engine barrier round in the NEFF epilogue
        with (
            tc.tile_pool(name="sb", bufs=1) as cpool,
            tc.tile_pool(name="ps", bufs=1, space="PSUM") as ps_pool,
        ):
            spool = wpool = qpool = xpool = epool = cpool
            ps_s = ps_av = ps_cp = ps_mp = ps_pool
            # ---- memsets first so nothing queues behind DMA descriptors.
            # warm goes on GpSimd (its init finishes first and its queue is
            # empty) so the PE warm-up can start ~2us earlier. ----
            warm = cpool.tile([128, 128], BF16)
            nc.gpsimd.memset(warm[:], 0.0)
            ones_f8 = cpool.tile([128, 1], F8)
            nc.gpsimd.memset(ones_f8[:], 1.0)
            ones_row = cpool.tile([1, TI], BF16)
            nc.gpsimd.memset(ones_row[:], 1.0)
            ebias_sb = cpool.tile([128, 1], F32)
            nc.gpsimd.memset(ebias_sb[:], EBIAS)
            corr_pad = []
            for c in range(3):
                cp = cpool.tile([KC, H + 2, W + 2], BF16, name=f"corr_pad{c}")
                nc.vector.memset(cp[:, 0, :], 0.0)
                nc.vector.memset(cp[:, H + 1, :], 0.0)
                nc.vector.memset(cp[:, 1:H + 1, 0:1], 0.0)
                nc.vector.memset(cp[:, 1:H + 1, W + 1:W + 2], 0.0)
                corr_pad.append(cp)
            # mask[r, ii, :] = 1 iff row r holds an ii-denominator partial
            # (ii0 partials live in rows {0,32}, ii1 in rows {64,96})
            mask = cpool.tile([128, 2, 128], BF16)
            nc.gpsimd.memset(mask[:], 0.0)
            for t in range(2):
                for ii in range(2):
                    r = 64 * ii + 32 * t
                    nc.vector.memset(mask[r:r + 1, ii, :], 1.0)

            # ---- input DMAs split across the two HWDGE rings: corr halves
            # on the Sync ring, weights/k/v on the Scalar ring, so the conv
            # critical path and the attention operands load in parallel ----
            wskT_sb = cpool.tile([KC, 27, D], BF16)
            k_sb = cpool.tile([D, HW], BF16)
            wvT2_sb = cpool.tile([128, NJP, 2, D], F8)
            def stg_copy(c, half):
                # second-half staging copies run on GpSimd: their DMAs
                # land mid-loop and would head-of-line block the DVE in
                # front of the conv q/resid copies
                stg = spool.tile([KC, HH * W], BF16, name="stg", bufs=3)
                nc.sync.dma_start(stg[:], corr[c * KC:(c + 1) * KC,
                                               half * HH * W:(half + 1) * HH * W])
                r0 = 1 + half * HH
                eng = nc.vector if half == 0 else nc.gpsimd
                eng.tensor_copy(
                    corr_pad[c][:, r0:r0 + HH, 1:W + 1],
                    stg.rearrange("p (h w) -> p h w", h=HH))

            # the conv's two input streams (weights + first corr halves)
            # interleave on the Sync ring in consumption order — the two
            # HWDGE rings share HBM bandwidth, so priority must be global,
            # not per-ring. k/vT2 ride the Scalar ring: S(0)/S(1) only
            # need the first k column-chunk, and the AV accumulation
            # starts at loop j=1, so vT2 beats k1/k2.
            nc.sync.dma_start(wskT_sb[:, 0:9, :], wskT[:, 0:9, :])
            stg_copy(0, 0)
            nc.sync.dma_start(wskT_sb[:, 9:18, :], wskT[:, 9:18, :])
            stg_copy(1, 0)
            nc.sync.dma_start(wskT_sb[:, 18:27, :], wskT[:, 18:27, :])
            stg_copy(2, 0)
            nc.scalar.dma_start(k_sb[:, 0:1024], k_in[:, 0:1024])
            nc.scalar.dma_start(wvT2_sb[:], wvT2[:])
            nc.scalar.dma_start(k_sb[:, 1024:2048], k_in[:, 1024:2048])
            nc.scalar.dma_start(k_sb[:, 2048:HW], k_in[:, 2048:HW])
            # preload the exp table set during the DMA wait (after the
            # Scalar-ring triggers so the table DMA doesn't delay them)
            tdummy = cpool.tile([1, 16], BF16)
            nc.scalar.activation(tdummy[:], warm[0:1, 0:16], AF.Exp)
            for c in range(3):
                stg_copy(c, 1)
            wp1T_sb = cpool.tile([D, D], BF16)
            nc.gpsimd.dma_start(wp1T_sb[:], wp1T[:])
            wf1T_sb = cpool.tile([D, D], BF16)
            nc.gpsimd.dma_start(wf1T_sb[:], wf1T[:])
            wf2T_sb = cpool.tile([D, D], BF16)
            nc.gpsimd.dma_start(wf2T_sb[:], wf2T[:])
            ident_sb = cpool.tile([D, D], BF16)
            nc.gpsimd.dma_start(ident_sb[:], ident[:])
            b_sk_sb = cpool.tile([1, D], BF16)
            nc.gpsimd.dma_start(b_sk_sb[:], b_sk[:])
            b_proj_sb = cpool.tile([1, D], BF16)
            nc.gpsimd.dma_start(b_proj_sb[:], b_proj[:])
            b_f1_sb = cpool.tile([D, 1], F32)
            nc.gpsimd.dma_start(b_f1_sb[:], b_f1[:])
            b_f2_sb = cpool.tile([1, D], BF16)
            nc.gpsimd.dma_start(b_f2_sb[:], b_f2[:])

            # HAM warm-up: keep PE busy while the first corr half lands
            ps_w = ps_cp.tile([128, 128], F32, name="ps_w", tag="c")
            for _ in range(54):
                nc.tensor.matmul(ps_w[:], warm[:], warm[:],
                                 start=True, stop=True)
            # dependency-free filler target for HAM-hold matmuls inside
            # the DMA-paced prologue conv ("m" slot is idle until j=12)
            ps_fill = ps_mp.tile([128, 128], F32, name="ps_fill", tag="m")

            def fill_pe(n):
                for _ in range(n):
                    nc.tensor.matmul(ps_fill[:], warm[:], warm[:],
                                     start=True, stop=True)

            def conv_gen(p, filler=False):
                """3x3 conv for i-tiles (2p, 2p+1), one [D,TI] psum bank,
                ii-serial. Also computes resid2 = conv + b_sk + b_proj
                + Wp1 @ q (the proj residual half, so the boundary only
                adds the normalized attention term). Yields None after
                each PE matmul, then yields [(resid2, q), ...] forever.
                filler=True (prologue pair only) emits dependency-free
                matmuls between weight chunks so the HAM clock gate never
                sees an idle window while the wskT DMA streams in."""
                outs = []
                for ii in range(2):
                    ps_c = ps_cp.tile([D, TI], F32, name="ps_c", tag="c")
                    y0 = (2 * p + ii) * RT
                    for c in range(3):
                        for t in range(9):
                            dy, dx = t // 3, t % 3
                            nc.tensor.matmul(
                                ps_c[:],
                                wskT_sb[:, c * 9 + t, :],
                                corr_pad[c][:, y0 + dy:y0 + dy + RT,
                                            dx:dx + W],
                                start=(c == 0 and t == 0),
                                stop=(c == 2 and t == 8 and not with_bias))
                            yield None
                        if filler and c < 2:
                            fill_pe(8)
                    if with_bias:
                        nc.tensor.matmul(ps_c[:], b_sk_sb[:], ones_row[:],
                                         start=False, stop=True)
                        yield None
                    q = qpool.tile([D, TI], BF16, name="q", tag="q", bufs=4)
                    nc.vector.tensor_copy(q[:], ps_c[:])
                    resid = qpool.tile([D, TI], F32, name="resid", tag="r", bufs=4)
                    nc.vector.tensor_copy(resid[:], ps_c[:])
                    yield None
                    yield None   # extra slack so the PE never waits on the
                    yield None   # DVE q-copy before the ps_r matmul
                    ps_r = ps_cp.tile([D, TI], F32, name="ps_r", tag="c")
                    nc.tensor.matmul(ps_r[:], wp1T_sb[:], q[:],
                                     start=True, stop=not with_bias)
                    if with_bias:
                        nc.tensor.matmul(ps_r[:], b_proj_sb[:], ones_row[:],
                                         start=False, stop=True)
                    yield None
                    resid2 = qpool.tile([D, TI], F32, name="resid2", tag="r2", bufs=4)
                    nc.vector.tensor_add(resid2[:], ps_r[:], resid[:])
                    outs.append((resid2, q))
                    yield None
                while True:
                    yield outs

            def run_conv(gen):
                while True:
                    r = next(gen)
                    if r is not None:
                        return r

            def s_pair(j, qs):
                t = ps_s.tile([128, 2, TI], F32, name="ps_sj", tag="s", bufs=2)
                for ii in range(2):
                    nc.tensor.matmul(t[:, ii, :],
                                     k_sb[:, j * 128:(j + 1) * 128],
                                     qs[ii][:], start=True, stop=True)
                return t

            def ones_half(ps_m, e2t, stop):
                """denominator partial sums for one e-tile (2 j-tiles):
                fp8 ones-matmuls col-packed 4-wide; ii0 -> rows {0,32},
                ii1 -> {64,96}. ps_m was zero-initialized by a start=True
                matmul, so always accumulate. The caller sets stop on the
                last e-tile so every row's accumulation chain closes."""
                for ii in range(2):
                    for jj in range(2):
                        r = 64 * ii + 32 * jj
                        nc.tensor.matmul(
                            ps_m[r:r + 1, :], ones_f8[:, 0:1],
                            e2t[:, jj, ii, :],
                            start=False, stop=stop,
                            skip_group_check=True,
                            tile_position=(0, r))

            def new_ps_m():
                """fresh denominator bank, zeroed by a 0-weights matmul so
                the mask-merge never touches stale PSUM garbage."""
                ps_m = ps_mp.tile([128, TI], F32, name="ps_m", tag="m")
                nc.tensor.matmul(ps_m[:], warm[:], k_sb[:, 0:TI],
                                 start=True, stop=True)
                return ps_m

            def merge_den(m4, ii):
                """broadcast mask-merged denominator into a psum bank."""
                ps_b = ps_mp.tile([128, TI], F32, name="ps_b", tag="m")
                nc.tensor.matmul(ps_b[:], mask[:, ii, :], m4[:],
                                 start=True, stop=True)
                return ps_b

            def recip_den(ps_b):
                rb = wpool.tile([128, TI], F32, name="rb", tag="rb", bufs=3)
                nc.vector.reciprocal_approx_fast(rb[:], ps_b[:])
                return rb

            def finish_x(av_sb, rb, resid2):
                t = wpool.tile([D, TI], F32, name="t", tag="t", bufs=3)
                nc.vector.tensor_mul(t[:], av_sb[:], rb[:])
                x_bf = xpool.tile([D, TI], BF16, name="x_bf", tag="xb", bufs=7)
                nc.vector.tensor_add(x_bf[:], t[:], resid2[:])
                return x_bf

            def ffn1(x_bf):
                ps_f1 = ps_av.tile([D, TI], F32, name="ps_f1", tag="av", bufs=2)
                nc.tensor.matmul(ps_f1[:], wf1T_sb[:], x_bf[:],
                                 start=True, stop=True)
                return ps_f1

            def gelu_of(ps_f1):
                h1 = wpool.tile([D, TI], BF16, name="h1", tag="h1", bufs=3)
                nc.scalar.activation(h1[:], ps_f1[:], AF.Gelu, bias=b_f1_sb[:])
                return h1

            def ffn2(x_bf, h1, use_ident):
                ps_f2 = ps_s.tile([D, TI], F32, name="ps_f2", tag="s", bufs=2)
                if use_ident:
                    nc.tensor.matmul(ps_f2[:], ident_sb[:], x_bf[:],
                                     start=True, stop=False)
                    nc.tensor.matmul(ps_f2[:], wf2T_sb[:], h1[:],
                                     start=False, stop=not with_bias)
                else:
                    nc.tensor.matmul(ps_f2[:], wf2T_sb[:], h1[:],
                                     start=True, stop=not with_bias)
                if with_bias:
                    nc.tensor.matmul(ps_f2[:], b_f2_sb[:], ones_row[:],
                                     start=False, stop=True)
                return ps_f2

            def out_tile(ps_f2, x_bf, i, use_ident):
                """PSUM->SBUF out copy + residual, then DMA. Even tiles got
                the residual via the identity matmul (ScalarE plain copy);
                odd tiles add it here on VectorE. DMA alternates rings."""
                o = wpool.tile([D, TI], F32, name="o", tag="o", bufs=3)
                if use_ident:
                    nc.scalar.copy(o[:], ps_f2[:])
                else:
                    nc.vector.tensor_add(o[:], ps_f2[:], x_bf[:])
                # even tiles exit via the Scalar ring right after their Act
                # copy; odd tiles via the Sync ring (idle at the tail)
                eng = nc.scalar if use_ident else nc.sync
                eng.dma_start(out[:, i * TI:(i + 1) * TI], o[:])

            # ---- prologue: conv pair 0 + S(0), S(1) ----
            rq = run_conv(conv_gen(0, filler=True))
            fill_pe(4)
            qpair = [rq[0][1], rq[1][1]]
            ps_s0 = s_pair(0, qpair)
            ps_s1 = s_pair(1, qpair)

            xs = [None] * NT
            prev = None              # (ps_m, ps_a, rq) of pair p-1
            for p in range(NP):
                last = p == NP - 1
                cgen = conv_gen(p + 1) if not last else None
                cdone = None
                qnext = None
                # boundary leftovers from pair p-1, injected into j=0..3 of
                # this pair's loop; the chain never blocks the PE because
                # conv matmuls are emitted first within each j.
                inject = []
                if prev is not None:
                    pm_ps, pav_ps, prq = prev
                    st = {}

                    def step0(pm_ps=pm_ps, pav_ps=pav_ps, st=st):
                        m4 = wpool.tile([128, TI], BF16, name="m4", tag="m4", bufs=3)
                        nc.vector.tensor_copy(m4[:], pm_ps[:])
                        avs = []
                        for ii in range(2):
                            a = qpool.tile([D, TI], F32, name="av_sb",
                                           tag="avs", bufs=4)
                            nc.vector.tensor_copy(a[:], pav_ps[ii][:])
                            avs.append(a)
                        st['m4'] = m4
                        st['av'] = avs
                        st['b0'] = merge_den(m4, 0)

                    def step1(st=st):
                        st['rb0'] = recip_den(st['b0'])
                        st['b1'] = merge_den(st['m4'], 1)

                    def step2(st=st, prq=prq, pp=p):
                        st['rb1'] = recip_den(st['b1'])
                        xs[2 * (pp - 1)] = finish_x(
                            st['av'][0], st['rb0'], prq[0][0])

                    def step3(st=st, prq=prq, pp=p):
                        xs[2 * (pp - 1) + 1] = finish_x(
                            st['av'][1], st['rb1'], prq[1][0])

                    inject = [step0, step1, step2, step3]

                ps_m = None
                ps_sj, ps_snx = ps_s0, ps_s1
                ps_a = None
                e2 = None
                e2_hist = []
                for j in range(NJ):
                    jp, jj = j // 2, j % 2
                    if jj == 0:
                        e2 = epool.tile([128, 2, 2, TI], F8, name="e2",
                                        tag="e", bufs=9)
                        e2_hist.append(e2)
                    # Act: the pacing instruction
                    nc.scalar.activation(e2[:, jj, :, :], ps_sj[:], AF.Exp,
                                         scale=SCALE, bias=ebias_sb[:])
                    # conv for the next pair: emitted first so the PE has
                    # dependency-free work while boundary copies settle
                    if cgen is not None:
                        for _ in range(3):
                            next(cgen)
                    # S prefetch / next-pair S tiles
                    if j < NJ - 2:
                        ps_sj, ps_snx = ps_snx, s_pair(j + 2, qpair)
                    elif j == NJ - 2:
                        ps_sj = ps_snx
                        if not last:
                            cdone = run_conv(cgen)
                            cgen = None
                            qnext = [cdone[0][1], cdone[1][1]]
                            ps_s0 = s_pair(0, qnext)
                    else:
                        if not last:
                            ps_s1 = s_pair(1, qnext)
                    # denominator groups: deferred so the 'm' bank is free
                    # of the previous boundary's merge chain; the last
                    # e-tile's half runs at the boundary (only 4 matmuls
                    # gated on the final exp)
                    if j in (12, 14, 16, 18, 20):
                        g4 = (j - 12) // 2
                        if ps_m is None:
                            ps_m = new_ps_m()
                        ones_half(ps_m, e2_hist[2 * g4], False)
                        ones_half(ps_m, e2_hist[2 * g4 + 1], False)
                    elif j == NJ - 2:
                        ones_half(ps_m, e2_hist[10], False)
                    # boundary injects (before AV so the previous pair's
                    # psum readers are emitted before this pair reuses the
                    # ps_av slots)
                    if inject and j < 4:
                        inject.pop(0)()
                    if jj == 1:
                        if jp == 0:
                            ps_a = [ps_av.tile([D, TI], F32,
                                               name=f"ps_a{ii}", tag="av",
                                               bufs=2)
                                    for ii in range(2)]
                        for ii in range(2):
                            nc.tensor.matmul(
                                ps_a[ii][:], wvT2_sb[:, jp, :, :],
                                e2[:, :, ii, :],
                                start=(jp == 0), stop=(jp == NJP - 1),
                                perf_mode=DR)
                # ---- boundary: finish denominators; evacuation and
                # normalize happen inside the next pair's loop (or tail) ----
                ones_half(ps_m, e2_hist[11], True)
                prev = (ps_m, ps_a, rq)
                if not last:
                    rq = cdone
                    qpair = qnext

            # ---- tail: pair-2 normalize + all 6 FFNs (gelus after all
            # exps: one table switch, issued first on Act so the table DMA
            # overlaps the merge/recip/normalize chain) ----
            pm_ps, pav_ps, rq2 = prev
            m4 = wpool.tile([128, TI], BF16, name="m4", tag="m4", bufs=3)
            nc.vector.tensor_copy(m4[:], pm_ps[:])
            av_t = []
            for ii in range(2):
                a = qpool.tile([D, TI], F32, name="av_sb", tag="avs", bufs=4)
                nc.vector.tensor_copy(a[:], pav_ps[ii][:])
                av_t.append(a)
            f1_0 = ffn1(xs[0])
            h0 = gelu_of(f1_0)
            f1_1 = ffn1(xs[1])
            b0 = merge_den(m4, 0)
            rb0 = recip_den(b0)
            h1 = gelu_of(f1_1)
            b1 = merge_den(m4, 1)
            rb1 = recip_den(b1)
            xs[4] = finish_x(av_t[0], rb0, rq2[0][0])
            xs[5] = finish_x(av_t[1], rb1, rq2[1][0])
            f2_0 = ffn2(xs[0], h0, True)
            out_tile(f2_0, xs[0], 0, True)
            f1_2 = ffn1(xs[2])
            h2 = gelu_of(f1_2)
            f2_1 = ffn2(xs[1], h1, False)
            out_tile(f2_1, xs[1], 1, False)
            f1_3 = ffn1(xs[3])
            h3 = gelu_of(f1_3)
            f2_2 = ffn2(xs[2], h2, True)
            out_tile(f2_2, xs[2], 2, True)
            f1_4 = ffn1(xs[4])
            h4 = gelu_of(f1_4)
            f2_3 = ffn2(xs[3], h3, False)
            out_tile(f2_3, xs[3], 3, False)
            f1_5 = ffn1(xs[5])
            h5 = gelu_of(f1_5)
            f2_4 = ffn2(xs[4], h4, True)
            out_tile(f2_4, xs[4], 4, True)
            f2_5 = ffn2(xs[5], h5, False)
            out_tile(f2_5, xs[5], 5, False)

    # populate .instr bytes for extended-inst ISA subclasses (the custom
    # DVE reciprocal) — raw Bass skips this Bacc.compile() pass and the
    # NEFF compiler fails with "ISA wrong length" without it
    mybir.codegen_inst_isa_subclasses(nc)
    _split_multi_waits(nc)
    return nc


_NC = {}


def _get_nc(with_bias=True):
    if with_bias not in _NC:
        _NC[with_bias] = build_nc(with_bias)
    return _NC[with_bias]


def _prep_core(corr, k, v, w_sk, b_sk, w_proj, b_proj, w_ffn1, b_ffn1,
               w_ffn2, b_ffn2):
    bf = ml_dtypes.bfloat16
    f8 = ml_dtypes.float8_e4m3
    wskT = np.empty((KC, 27, D), dtype=bf)
    for c in range(3):
        for t in range(9):
            dy, dx = t // 3, t % 3
            wskT[:, c * 9 + t, :] = \
                w_sk[:, c * KC:(c + 1) * KC, dy, dx].T.astype(bf)
    # fold the attention half of the proj into v:
    # w_proj @ concat(av, resid) = Wp0 @ av + Wp1 @ resid, and
    # Wp0 @ (V E) == (Wp0 V) @ E, so quantize Wp0 V to fp8 instead of V.
    wp = w_proj.reshape(D, 2 * D)
    wv = wp[:, :D] @ v.reshape(D, HW)                     # [D, HW] f32
    wvT = wv.T.reshape(NJ, 128, D).transpose(1, 0, 2)
    wvT2 = np.ascontiguousarray(wvT).astype(f8).reshape(128, NJP, 2, D)
    return {
        "corr": corr.reshape(CIN, HW).astype(bf),
        "k": k.reshape(D, HW).astype(bf),
        "wvT2": wvT2,
        "wskT": wskT,
        "b_sk": b_sk.reshape(1, D).astype(bf),
        "wp1T": np.ascontiguousarray(wp[:, D:].T).astype(bf),
        "b_proj": b_proj.reshape(1, D).astype(bf),
        "wf1T": np.ascontiguousarray(w_ffn1.reshape(D, D).T).astype(bf),
        "b_f1": b_ffn1.reshape(D, 1).astype(np.float32),
        "wf2T": np.ascontiguousarray(w_ffn2.reshape(D, D).T).astype(bf),
        "b_f2": b_ffn2.reshape(1, D).astype(bf),
        "ident": np.eye(D, dtype=bf),
    }


def make_in_maps(corr, k, v, w_sk, b_sk, w_proj, b_proj, w_ffn1, b_ffn1,
                 w_ffn2, b_ffn2):
    corr = np.asarray(corr, dtype=np.float32)
    k = np.asarray(k, dtype=np.float32)
    v = np.asarray(v, dtype=np.float32)
    return [
        _prep_core(corr[i], k[i], v[i], np.asarray(w_sk, np.float32),
                   np.asarray(b_sk, np.float32),
                   np.asarray(w_proj, np.float32),
                   np.asarray(b_proj, np.float32),
                   np.asarray(w_ffn1, np.float32),
                   np.asarray(b_ffn1, np.float32),
                   np.asarray(w_ffn2, np.float32),
                   np.asarray(b_ffn2, np.float32))
        for i in range(N)
    ]


def kernel(corr, k, v, w_sk, b_sk, w_proj, b_proj, w_ffn1, b_ffn1,
           w_ffn2, b_ffn2):
    with_bias = bool(np.any(np.asarray(b_proj)) or np.any(np.asarray(b_ffn2))
                     or np.any(np.asarray(b_sk)))
    nc = _get_nc(with_bias)
    in_maps = make_in_maps(corr, k, v, w_sk, b_sk, w_proj, b_proj,
                           w_ffn1, b_ffn1, w_ffn2, b_ffn2)
    res = run_bass_kernel_spmd(nc, in_maps, list(range(N)))
    out = np.stack([res.results[i]["out"].reshape(D, H, W) for i in range(N)])
    return out.astype(np.float32)
